# revision 49
# baseline (speedup 1.0000x reference)
"""BiGraphSAGEDecoder Trainium2 kernel (v2 — restructured).

Sharding: 8 cores = 4 batches x {up-path, down-path}. One SPMD bass program;
up/down asymmetry handled by data (down cores get host-transposed matrices).

Key restructurings vs v1:
  - mm1 emits s^T directly (lhsT = h blocks, rhs = prod strips), enabling
    the reassociation x' = inv @ (s @ Wc): the big [N,N] matmul contracts
    against 256 cols instead of din=768.
  - layer 2 never computes s: x2 = ivr @ prod^T @ h is folded as
    u = ivr @ prodT (prodT streamed from host-transposed adj/wadj),
    w = u^T, x2 = w^T @ h.
  - full bias chunk computed locally (no bias in the exchange); AllGather
    moves only the 256-col own chunk per rank, in fp16.
  - dtypes: adj/wadj/inv fp16 in DRAM, prod fp16, h fp16, y fp16,
    sT fp32r, weights fp32r, psum fp32. Validated max rel err 1.9e-3
    vs fp32 reference (tolerance 2e-2).

Math per layer (per core, its path):
  prod = adj .* wadj_baked                  (DVE, fp16)
  sT   = (prod^T @ h)^T  via lhsT=h         (PE)
  y    = s @ Wc                             (PE, lhsT = sT blocks)
  own  = inv @ y                            (PE, lhsT = invT blocks)
  bias = h @ Wb (full)                      (PE, via PE-transposed h)
  exchange own chunks (2-rank AllGather, two halves, fp16)
  h' = lrelu(cat / max(||cat||, eps))       (DVE/scalar/gpsimd)
"""

import os
import sys
import types
import contextlib

sys.path.insert(0, "/opt/trn_rl_repo")

import numpy as np

import concourse.bass as bass
import concourse.tile as tile
from concourse import mybir, bacc
from concourse.mybir import AxisListType
from concourse.masks import make_identity
from concourse.bass_utils import run_bass_kernel_spmd

FP = mybir.dt.float32
FPR = mybir.dt.float32r
F16 = mybir.dt.float16
F8 = mybir.dt.float8e4
AF = mybir.ActivationFunctionType
ALU = mybir.AluOpType

# ---------------------------------------------------------------------------
# Environment patches (required for this container's toolchain)
# ---------------------------------------------------------------------------


def install_ntff_shim():
    """antenv.axon_hooks is absent in this image; provide it so trace=True
    profiling works (used by test.py, harmless otherwise)."""
    try:
        import antenv.axon_hooks  # noqa: F401
        return
    except ImportError:
        pass
    try:
        import antenv
    except ImportError:
        return
    mod = types.ModuleType("antenv.axon_hooks")
    _holder = {"hook": None}
    mod.set_axon_ntff_profile_hook = lambda h: _holder.__setitem__("hook", h)
    mod.get_axon_ntff_profile_hook = lambda: _holder["hook"]
    sys.modules["antenv.axon_hooks"] = mod
    antenv.axon_hooks = mod
    try:
        from trn_agent_boot.trn_boot import _ntff_profile_via_ctypes

        hook = _ntff_profile_via_ctypes("/opt/axon/libaxon_pjrt.so")
        if hook is not None:
            mod.set_axon_ntff_profile_hook(hook)
    except Exception:
        pass


install_ntff_shim()

if os.environ.get("KGSD_LDW_OPT", "0") != "0":
    # let walrus dedup back-to-back LDWEIGHTS
    import concourse.bass_utils as _bu
    _orig_run_command = _bu.run_command

    def _patched_run_command(argv, **kw):
        argv = ["--enable-ldw-opt=true" if a == "--enable-ldw-opt=false"
                else a for a in argv]
        return _orig_run_command(argv, **kw)

    _bu.run_command = _patched_run_command

# ---------------------------------------------------------------------------
# Problem constants
# ---------------------------------------------------------------------------

N_FULL = 2048
B = 4
P = 128
DOUT = 256     # per-path cat chunk width (also bias width)
DEC = 128
DINS = (256, 768, 768)   # per-layer input dims
D3 = 768
EPS = 1e-12
LEAK = 0.1
JB = 512       # mm1 / x' column block (one PSUM bank of fp32)


# ---------------------------------------------------------------------------
# Program builder
# ---------------------------------------------------------------------------


def build_program(n_cores: int, N: int = N_FULL, stop_phase: int = 99,
                  fused_lrelu: bool = True):
    """Build the SPMD bass program.

    stop_phase (debug): 1=x load, 2=l0 mm1, 3=l0 y, 4=l0 x', 5=l0 done,
    6=l1 done, 7=full.
    """
    NT = N // P              # 128-row tiles
    NJB = N // JB            # mm1/x' column blocks
    HALF = NT // 2
    NQ = max(N // JB, 1)     # aw quarter count (512-wide pairs)

    nc = bacc.Bacc("TRN2", target_bir_lowering=False, debug=False,
                   num_devices=n_cores)

    # --- DRAM I/O ---
    x_d = nc.dram_tensor("x", [N, DINS[0]], F16, kind="ExternalInput")
    adj_d = nc.dram_tensor("adj", [N, N], F16, kind="ExternalInput")
    wa_d = [nc.dram_tensor(f"wa{l}", [N, N], F8, kind="ExternalInput")
            for l in range(2)]
    adjT_d = nc.dram_tensor("adjT", [N, N], F16, kind="ExternalInput")
    waT_d = nc.dram_tensor("waT", [N, N], F8, kind="ExternalInput")
    invT_d = nc.dram_tensor("invT", [N, N], F16, kind="ExternalInput")
    ivrT_d = nc.dram_tensor("ivrT", [N, 2], F16, kind="ExternalInput")
    wc_d = [nc.dram_tensor(f"w{l}c", [DINS[l], DOUT], FP, kind="ExternalInput")
            for l in range(3)]
    wb_d = [nc.dram_tensor(f"w{l}b", [DINS[l], DOUT], FP, kind="ExternalInput")
            for l in range(3)]
    p1_d = nc.dram_tensor("p1", [D3, DEC], FP, kind="ExternalInput")
    p2_d = nc.dram_tensor("p2", [DEC, DEC], FP, kind="ExternalInput")
    y_d = nc.dram_tensor("ypred", [1, 1], FP, kind="ExternalOutput")
    dbg_d = (nc.dram_tensor("dbg", [N, D3], FP, kind="ExternalOutput")
             if stop_phase < 99 else None)

    groups = [[i, i + 1] for i in range(0, n_cores, 2)]

    with tile.TileContext(nc) as tc:
        with contextlib.ExitStack() as ctx:
            const_p = ctx.enter_context(tc.tile_pool(name="const", bufs=1))
            h_p = ctx.enter_context(tc.tile_pool(name="h", bufs=1))
            sT_p = ctx.enter_context(tc.tile_pool(name="sT", bufs=1))
            y_p = ctx.enter_context(tc.tile_pool(name="y", bufs=1))
            aw_p = ctx.enter_context(tc.tile_pool(name="aw", bufs=4))
            prod_p = ctx.enter_context(tc.tile_pool(name="prod", bufs=4))
            inv_p = ctx.enter_context(tc.tile_pool(name="invs", bufs=2))
            hT_p = ctx.enter_context(tc.tile_pool(name="hT", bufs=2))
            stg_p = ctx.enter_context(tc.tile_pool(name="stg", bufs=2))
            norm_p = ctx.enter_context(tc.tile_pool(name="norm", bufs=2))
            misc_p = ctx.enter_context(tc.tile_pool(name="misc", bufs=1))
            psum_p = ctx.enter_context(
                tc.tile_pool(name="psum", bufs=8, space="PSUM"))
            dram_p = ctx.enter_context(
                tc.tile_pool(name="dram", bufs=2, space="DRAM"))

            ident = const_p.tile([P, P], FP, tag="ident")
            make_identity(nc, ident)
            ident16 = const_p.tile([P, P], F16, tag="ident16")
            nc.scalar.copy(ident16[:], ident[:])

            # --- constant tiles (loads deferred; see _load_w) ---
            wc_t, wb_t = [], []
            for l in range(3):
                ND = DINS[l] // P
                wct = const_p.tile([P, ND * DOUT], FPR, tag=f"wc{l}")
                wbt = const_p.tile([P, ND * DOUT], FPR, tag=f"wb{l}")
                wc_t.append(wct)
                wb_t.append(wbt)

            def _load_w(l):
                ND = DINS[l] // P
                for d in range(ND):
                    nc.scalar.dma_start(
                        wc_t[l][:, d * DOUT:(d + 1) * DOUT],
                        wc_d[l].ap()[d * P:(d + 1) * P, :].bitcast(FPR))
                    nc.scalar.dma_start(
                        wb_t[l][:, d * DOUT:(d + 1) * DOUT],
                        wb_d[l].ap()[d * P:(d + 1) * P, :].bitcast(FPR))

            _load_w(0)
            p1_t = const_p.tile([P, (D3 // P) * DEC], FP, tag="p1")
            p2_t = const_p.tile([P, DEC], FP, tag="p2")
            ivrT_t = const_p.tile([P, NT, 2], F16, tag="ivrT")
            nc.scalar.dma_start(
                ivrT_t[:],
                ivrT_d.ap().rearrange("(a p) c -> p a c", p=P))

            # --- h double generation (fp16) ---
            h_e = [h_p.tile([P, D3], F16, tag=f"he{k}", name="h_e")
                   for k in range(NT)]
            h_o = [h_p.tile([P, D3], F16, tag=f"ho{k}", name="h_o")
                   for k in range(NT)]
            # x loads are interleaved with the first mm1 strip loads below

            sT = [sT_p.tile([P, N], FPR, tag=f"sT{d}", name="sT")
                  for d in range(6)]
            y_t = [y_p.tile([P, DOUT], F16, tag=f"y{j}", name="y_t")
                   for j in range(NT)]
            w_sb = misc_p.tile([P, 2 * NT], F16, tag="w_sb")

            def _dump_and_done(src_ap):
                y_sb0 = misc_p.tile([1, 1], FP, tag="y_dbg")
                nc.vector.tensor_copy(y_sb0[:], src_ap)
                nc.sync.dma_start(y_d.ap(), y_sb0[:])

            # l2 prefold bursts: split quarters between the two boundaries;
            # steps are interleaved into the bias loops (one step per it)
            burst_q = [list(range(NQ))[:(NQ + 1) // 2],
                       list(range(NQ))[(NQ + 1) // 2:]]
            pu_live = {}

            def burst_step(qg, jj):
                if jj == 0:
                    pu_live[qg] = psum_p.tile([P, JB], FP, tag="ps",
                                              name="pu")
                pu = pu_live[qg]
                rsl = slice(jj * 2 * P, (jj + 1) * 2 * P)
                csl = slice(qg * JB, (qg + 1) * JB)
                a2t = aw_p.tile([P, 2, JB], F16, tag="adj")
                nc.scalar.dma_start(
                    a2t[:], adjT_d.ap()[rsl, csl]
                    .rearrange("(a p) c -> p a c", p=P))
                w8t = aw_p.tile([P, 2, JB], F8, tag="wa8", bufs=2)
                nc.scalar.dma_start(
                    w8t[:], waT_d.ap()[rsl, csl]
                    .rearrange("(a p) c -> p a c", p=P))
                w2t = aw_p.tile([P, 2, JB], F16, tag="wa")
                nc.scalar.copy(w2t[:], w8t[:])
                pr = prod_p.tile([P, 2, JB], F16, tag="prod")
                nc.gpsimd.tensor_tensor(pr[:], a2t[:], w2t[:], ALU.mult)
                for a in range(2):
                    j = jj * 2 + a
                    nc.tensor.matmul(pu[0:2, :], ivrT_t[:, j, :],
                                     pr[:, a, :], start=(j == 0),
                                     stop=(j == NT - 1))
                if jj == NT // 2 - 1:
                    usb = misc_p.tile([2, JB], F16, tag="usb")
                    nc.scalar.copy(usb[:], pu[0:2, :])
                    for kk in range(JB // P):
                        k = qg * (JB // P) + kk
                        if k >= NT:
                            break
                        ptw = psum_p.tile([P, 2], F16, tag="ps", name="ptw")
                        nc.tensor.transpose(
                            ptw[:], usb[:, kk * P:(kk + 1) * P],
                            ident16[0:2, 0:2])
                        nc.vector.tensor_copy(w_sb[:, 2 * k:2 * k + 2],
                                              ptw[:])

            if stop_phase <= 1:
                _dump_and_done(h_e[0][0:1, 0:1])

            hcur, hnxt = h_e, h_o
            n_layers = 0 if stop_phase <= 1 else (2 if stop_phase <= 6 else 2)
            for l in range(2):
                if stop_phase <= 1:
                    break
                din = DINS[l]
                ND = din // P

                # ---- mm1: sT = (prod^T @ h)^T ----
                for jb in range(NJB):
                    pss = [psum_p.tile([P, JB], FP, tag="ps", name="pmm1")
                           for _ in range(ND)]
                    for kk in range(NT // 2):
                        if l == 0 and jb == 0:
                            for a in range(2):
                                kt = kk * 2 + a
                                nc.sync.dma_start(
                                    h_e[kt][:, 0:DINS[0]],
                                    x_d.ap()[kt * P:(kt + 1) * P, :])
                        rsl = slice(kk * 2 * P, (kk + 1) * 2 * P)
                        csl = slice(jb * JB, (jb + 1) * JB)
                        adt = aw_p.tile([P, 2, JB], F16, tag="adj")
                        nc.sync.dma_start(
                            adt[:], adj_d.ap()[rsl, csl]
                            .rearrange("(a p) c -> p a c", p=P))
                        wa8 = aw_p.tile([P, 2, JB], F8, tag="wa8", bufs=2)
                        nc.sync.dma_start(
                            wa8[:], wa_d[l].ap()[rsl, csl]
                            .rearrange("(a p) c -> p a c", p=P))
                        wat = aw_p.tile([P, 2, JB], F16, tag="wa")
                        nc.vector.tensor_copy(wat[:], wa8[:])
                        pr = prod_p.tile([P, 2, JB], F16, tag="prod")
                        nc.vector.tensor_tensor(pr[:], adt[:], wat[:],
                                                ALU.mult)
                        for a in range(2):
                            k = kk * 2 + a
                            for d in range(ND):
                                nc.tensor.matmul(
                                    pss[d][:], hcur[k][:, d * P:(d + 1) * P],
                                    pr[:, a, :], start=(k == 0),
                                    stop=(k == NT - 1))
                    for d in range(ND):
                        dst = sT[d][:, jb * JB:(jb + 1) * JB]
                        if d % 2 == 0:
                            nc.scalar.copy(dst, pss[d][:])
                        else:
                            nc.vector.tensor_copy(dst, pss[d][:])

                if stop_phase == 2 and l == 0:
                    _dump_and_done(sT[0][0:1, 0:1])
                    break

                # ---- y = s @ Wc  (fp16) ----
                for j in range(NT):
                    py = psum_p.tile([P, DOUT], FP, tag="ps", name="py")
                    for d in range(ND):
                        nc.tensor.matmul(
                            py[:], sT[d][:, j * P:(j + 1) * P],
                            wc_t[l][:, d * DOUT:(d + 1) * DOUT],
                            start=(d == 0), stop=(d == ND - 1))
                    nc.scalar.copy(y_t[j][:], py[:])

                if stop_phase == 3 and l == 0:
                    _dump_and_done(y_t[0][0:1, 0:1])
                    break

                # ---- x' = inv @ y : own cat chunk ----
                pxs = [psum_p.tile([P, JB], FP, tag="ps", name="px")
                       for _ in range((NT + 1) // 2)]
                for px in pxs:
                    nc.vector.memset(px[:], 0.0)
                for j in range(NT):
                    ivt = inv_p.tile([P, N], F16, tag="inv", bufs=3)
                    nc.sync.dma_start(ivt[:],
                                      invT_d.ap()[j * P:(j + 1) * P, :])
                    for i in range(NT):
                        px = pxs[i // 2]
                        sl = slice((i % 2) * DOUT, (i % 2 + 1) * DOUT)
                        nc.tensor.matmul(
                            px[:, sl], ivt[:, i * P:(i + 1) * P], y_t[j][:],
                            start=False, stop=(j == NT - 1),
                            skip_group_check=True)

                if stop_phase == 4 and l == 0:
                    _dump_and_done(y_t[0][0:1, 0:1])
                    break

                # ---- stage + AllGather (two halves) ----
                stage_d = [dram_p.tile([N // 2, DOUT], F16, tag=f"stg{hh}",
                                       name="stage_d") for hh in range(2)]
                GW = min(4, HALF)  # i-blocks per staged DMA
                for g in range(NT // GW):
                    st = stg_p.tile([P, GW, DOUT], F16, tag="stg")
                    for a2 in range(GW // 2):
                        nc.scalar.copy(
                            st[:, 2 * a2:2 * a2 + 2, :],
                            pxs[(g * GW) // 2 + a2][:])
                    hh, go = divmod(g, HALF // GW)
                    nc.scalar.dma_start(
                        stage_d[hh][go * GW * P:(go + 1) * GW * P, :]
                        .rearrange("(a p) c -> p a c", p=P),
                        st[:])
                ag_t = []
                for hh in range(2):
                    agt = dram_p.tile([2, N // 2, DOUT], F16, tag=f"ag{hh}",
                                      name="ag_t")
                    nc.gpsimd.collective_compute(
                        "AllGather", ALU.bypass, replica_groups=groups,
                        ins=[stage_d[hh].opt()], outs=[agt.opt()])
                    ag_t.append(agt)

                # ---- weight prefetch for later phases (scalar queue) ----
                if l == 0:
                    _load_w(1)
                else:
                    _load_w(2)
                    for d in range(D3 // P):
                        nc.scalar.dma_start(p1_t[:, d * DEC:(d + 1) * DEC],
                                            p1_d.ap()[d * P:(d + 1) * P, :])
                    nc.scalar.dma_start(p2_t[:], p2_d.ap())

                # ---- bias = h @ Wb (full, local) -> h' cols 512:768 ----
                # (one l2-prefold burst step interleaved per it)
                bsteps = [(qg, jj) for qg in burst_q[l]
                          for jj in range(NT // 2)]
                for it in range(NT):
                    pb = psum_p.tile([P, DOUT], FP, tag="ps", name="pb")
                    for dd in range(ND // 2):
                        ptr = psum_p.tile([P, 2 * P], F16, tag="ps",
                                          name="ptr")
                        nc.tensor.matmul(
                            ptr[:, 0:P], hcur[it][:, 2 * dd * P:
                                                  (2 * dd + 1) * P],
                            ident16[:], is_transpose=True,
                            start=True, stop=False, skip_group_check=True)
                        nc.tensor.matmul(
                            ptr[:, P:2 * P],
                            hcur[it][:, (2 * dd + 1) * P:(2 * dd + 2) * P],
                            ident16[:], is_transpose=True,
                            start=False, stop=True, skip_group_check=True)
                        hTt = hT_p.tile([P, 2 * P], FPR, tag="hT")
                        nc.vector.tensor_copy(hTt[:], ptr[:])
                        for a in range(2):
                            d = 2 * dd + a
                            nc.tensor.matmul(
                                pb[:], hTt[:, a * P:(a + 1) * P],
                                wb_t[l][:, d * DOUT:(d + 1) * DOUT],
                                start=(d == 0), stop=(d == ND - 1))
                    nc.scalar.copy(hnxt[it][:, 2 * DOUT:3 * DOUT], pb[:])
                    if it < len(bsteps):
                        burst_step(*bsteps[it])

                # ---- leftover burst steps (if NT < steps) ----
                for st_ in bsteps[NT:]:
                    burst_step(*st_)

                # ---- assemble + normalize + lrelu -> h' ----
                # (for l1: x2 = w^T @ h matmuls trail each tile's norm)
                if l == 1 and stop_phase > 6:
                    psA = psum_p.tile([P, JB], FP, tag="ps", name="psA")
                    psB = psum_p.tile([P, JB], FP, tag="ps", name="psB")
                for it in range(NT):
                    hh, io = divmod(it, HALF)
                    ag = ag_t[hh]
                    ht = hnxt[it]
                    nc.gpsimd.dma_start(
                        ht[:, 0:2 * DOUT],
                        ag[:, io * P:(io + 1) * P, :]
                        .rearrange("r p c -> p r c"))
                    if stop_phase == 4.75 and l == 0:
                        dbf = misc_p.tile([P, D3], FP, tag="dbf", bufs=2)
                        nc.vector.tensor_copy(dbf[:], ht[:])
                        nc.sync.dma_start(
                            dbg_d.ap()[it * P:(it + 1) * P, :], dbf[:])
                        continue
                    sqs = norm_p.tile([P, D3], F16, tag="sq")
                    ssq = norm_p.tile([P, 1], FP, tag="ssq")
                    if it % 2 == 0:
                        nc.scalar.activation(sqs[:], ht[:], AF.Square,
                                             accum_out=ssq[:])
                    else:
                        nc.vector.scalar_tensor_tensor(
                            sqs[:], ht[:], 1.0, ht[:], ALU.mult, ALU.mult,
                            accum_out=ssq[:])
                    nrm = norm_p.tile([P, 1], FP, tag="nrm")
                    nc.scalar.activation(nrm[:], ssq[:], AF.Sqrt)
                    rn = norm_p.tile([P, 1], FP, tag="rn")
                    nc.vector.reciprocal(rn[:], nrm[:])
                    if fused_lrelu and it % 2 == 0:
                        nc.scalar.activation(ht[:], ht[:], AF.Lrelu,
                                             scale=rn[:], alpha=LEAK)
                    else:
                        nc.vector.tensor_scalar(ht[:], ht[:], rn[:], None,
                                                ALU.mult)
                        nc.vector.scalar_tensor_tensor(
                            ht[:], ht[:], LEAK, ht[:], ALU.mult, ALU.max)
                    if l == 1 and stop_phase > 6:
                        nc.tensor.matmul(psA[0:2, :],
                                         w_sb[:, 2 * it:2 * it + 2],
                                         ht[:, 0:JB], start=(it == 0),
                                         stop=(it == NT - 1))
                        nc.tensor.matmul(psB[0:2, 0:D3 - JB],
                                         w_sb[:, 2 * it:2 * it + 2],
                                         ht[:, JB:D3], start=(it == 0),
                                         stop=(it == NT - 1))

                if stop_phase == 4.75 and l == 0:
                    break
                hcur, hnxt = hnxt, hcur
                if stop_phase == 5 and l == 0:
                    _dump_and_done(hcur[0][0:1, 0:1])
                    for it in range(NT):
                        dbf = misc_p.tile([P, D3], FP, tag="dbf", bufs=2)
                        nc.vector.tensor_copy(dbf[:], hcur[it][:])
                        nc.sync.dma_start(
                            dbg_d.ap()[it * P:(it + 1) * P, :], dbf[:])
                    break

            do_tail = stop_phase > 6
            if do_tail:
                # ---- l2 tail (x2 psums already accumulated in norm loop) ----
                x2sb = misc_p.tile([2, D3], F16, tag="x2sb")
                nc.scalar.copy(x2sb[:, 0:JB], psA[0:2, :])
                nc.scalar.copy(x2sb[:, JB:D3], psB[0:2, 0:D3 - JB])

                # own2 = x2 @ Wc2
                pc2 = psum_p.tile([P, DOUT], FP, tag="ps", name="pc2")
                for d in range(D3 // P):
                    ptx = psum_p.tile([P, 2], F16, tag="ps", name="ptx")
                    nc.tensor.transpose(
                        ptx[:], x2sb[:, d * P:(d + 1) * P],
                        ident16[0:2, 0:2])
                    x2T = hT_p.tile([P, 2], FPR, tag="x2T")
                    nc.vector.tensor_copy(x2T[:], ptx[:])
                    nc.tensor.matmul(pc2[0:2, :], x2T[:],
                                     wc_t[2][:, d * DOUT:(d + 1) * DOUT],
                                     start=(d == 0), stop=(d == D3 // P - 1))

                # stage2 + AG2 launched before bias2 (AG flies during bias2)
                stg2 = stg_p.tile([2, DOUT], F16, tag="stg")
                nc.scalar.copy(stg2[:], pc2[0:2, :])
                st2d = dram_p.tile([2, DOUT], F16, tag="stg2")
                nc.scalar.dma_start(st2d[:], stg2[:])
                ag2 = dram_p.tile([2, 2, DOUT], F16, tag="ag2")
                nc.gpsimd.collective_compute(
                    "AllGather", ALU.bypass, replica_groups=groups,
                    ins=[st2d.opt()], outs=[ag2.opt()])

                # bias2 = hdrug @ Wb2 (hdrug via DRAM bounce, post-norm rows)
                bounce = dram_p.tile([2, D3], F16, tag="bounce")
                nc.sync.dma_start(bounce[:], hcur[NT - 1][P - 2:P, :])
                hdr = misc_p.tile([2, D3], F16, tag="hdrug")
                nc.sync.dma_start(hdr[:], bounce[:])
                pb2 = psum_p.tile([P, DOUT], FP, tag="ps", name="pb2")
                for d in range(D3 // P):
                    ptr = psum_p.tile([P, 2], F16, tag="ps", name="ptr2")
                    nc.tensor.transpose(ptr[:], hdr[:, d * P:(d + 1) * P],
                                        ident16[0:2, 0:2])
                    hTt = hT_p.tile([P, 2], FPR, tag="x2T")
                    nc.vector.tensor_copy(hTt[:], ptr[:])
                    nc.tensor.matmul(pb2[0:2, :], hTt[:],
                                     wb_t[2][:, d * DOUT:(d + 1) * DOUT],
                                     start=(d == 0), stop=(d == D3 // P - 1))

                # assemble + norm
                asm2 = stg_p.tile([2, 2 * DOUT], F16, tag="asm2")
                nc.sync.dma_start(asm2[:, 0:DOUT], ag2[0, :, :])
                nc.sync.dma_start(asm2[:, DOUT:2 * DOUT], ag2[1, :, :])
                dr = misc_p.tile([2, D3], FP, tag="drug")
                nc.vector.tensor_copy(dr[:, 0:2 * DOUT], asm2[:])
                nc.vector.tensor_copy(dr[:, 2 * DOUT:D3], pb2[0:2, :])
                sq = norm_p.tile([2, D3], FP, tag="sq2")
                ssq = norm_p.tile([2, 1], FP, tag="ssq2")
                nc.vector.tensor_tensor(sq[:], dr[:], dr[:], ALU.mult)
                nc.vector.tensor_reduce(ssq[:], sq[:], AxisListType.X,
                                        ALU.add)
                nrm = norm_p.tile([2, 1], FP, tag="nrm2")
                nc.scalar.activation(nrm[:], ssq[:], AF.Sqrt)
                nc.vector.tensor_scalar_max(nrm[:], nrm[:], EPS)
                rn = norm_p.tile([2, 1], FP, tag="rn2")
                nc.vector.reciprocal(rn[:], nrm[:])
                nc.vector.tensor_scalar(dr[:], dr[:], rn[:], None, ALU.mult)
                nc.scalar.mul(sq[:], dr[:], LEAK)
                nc.vector.tensor_max(dr[:], dr[:], sq[:])

                # ---- head: ypred = (a P1 P2) . (b P1) ----
                ND3 = D3 // P
                dT = misc_p.tile([P, ND3 * 2], FP, tag="dT")
                for d in range(ND3):
                    pt = psum_p.tile([P, 2], FP, tag="ps", name="phd")
                    nc.tensor.transpose(pt[:], dr[:, d * P:(d + 1) * P],
                                        ident[0:2, 0:2])
                    nc.vector.tensor_copy(dT[:, d * 2:(d + 1) * 2], pt[:])
                pw = psum_p.tile([P, 2], FP, tag="ps", name="pw")
                for d in range(ND3):
                    nc.tensor.matmul(pw[:], p1_t[:, d * DEC:(d + 1) * DEC],
                                     dT[:, d * 2:(d + 1) * 2],
                                     start=(d == 0), stop=(d == ND3 - 1))
                w_hd = misc_p.tile([P, 2], FP, tag="w_hd")
                nc.vector.tensor_copy(w_hd[:], pw[:])
                ptt = psum_p.tile([P, 1], FP, tag="ps", name="ptt")
                nc.tensor.matmul(ptt[:], p2_t[:], w_hd[:, 0:1], start=True,
                                 stop=True)
                t_sb = misc_p.tile([P, 1], FP, tag="t_sb")
                nc.vector.tensor_copy(t_sb[:], ptt[:])
                py_ = psum_p.tile([1, 1], FP, tag="ps", name="pyf")
                nc.tensor.matmul(py_[:], t_sb[:], w_hd[:, 1:2], start=True,
                                 stop=True)
                y_sb = misc_p.tile([1, 1], FP, tag="y_sb")
                nc.vector.tensor_copy(y_sb[:], py_[:])
                nc.sync.dma_start(y_d.ap(), y_sb[:])
            elif stop_phase > 5:
                _dump_and_done(hcur[0][0:1, 0:1])

    nc.compile()
    return nc


# ---------------------------------------------------------------------------
# Host-side input prep
# ---------------------------------------------------------------------------

def _pack_quarters(a16, w16, N):
    """Interleave 512-col quarters: [a[:,q] | w[:,q]] -> [N, 2N] fp16."""
    NQ = max(N // JB, 1)
    q = min(JB, N)
    out = np.empty((N, 2 * N), dtype=np.float16)
    for i in range(NQ):
        out[:, i * 2 * q:i * 2 * q + q] = a16[:, i * q:(i + 1) * q]
        out[:, i * 2 * q + q:(i + 1) * 2 * q] = w16[:, i * q:(i + 1) * q]
    return out


def make_in_maps(inputs: dict, n_cores: int, N: int = None):
    """Per-core input dicts. Core 2b = up path of batch b, 2b+1 = down."""
    if N is None:
        N = np.asarray(inputs["adj"]).shape[-1]
    f32c = lambda a: np.ascontiguousarray(np.asarray(a, dtype=np.float32))
    f16c = lambda a: np.ascontiguousarray(np.asarray(a).astype(np.float16))

    def bake_mask(w):
        w = np.array(w, dtype=np.float32)
        w[-2:, :] = 1.0
        w[:, -2:] = 1.0
        return w

    maps = []
    for c in range(n_cores):
        b, down = divmod(c, 2)
        if not down:
            A = np.asarray(inputs["adj"][b])
            IV = np.asarray(inputs["up_inv_deg"][b])
            was = [bake_mask(inputs[f"l{l}_up_adj_w"]) for l in range(3)]
            wcs = [inputs[f"l{l}_up_w"] for l in range(3)]
        else:
            A = np.asarray(inputs["adj"][b]).T
            IV = np.asarray(inputs["down_inv_deg"][b])
            was = [bake_mask(inputs[f"l{l}_down_adj_w"]).T for l in range(3)]
            wcs = [inputs[f"l{l}_down_w"] for l in range(3)]
        import ml_dtypes
        f8c = lambda a: np.ascontiguousarray(
            np.asarray(a).astype(ml_dtypes.float8_e4m3))
        A16 = f16c(A)
        m = {
            "x": f16c(inputs["x"][b]),
            "adj": A16,
            "adjT": np.ascontiguousarray(A16.T),
            "invT": f16c(IV.T),
            "ivrT": f16c(IV[-2:, :].T),
            "p1": f32c(inputs["parameter1"]),
            "p2": f32c(inputs["parameter2"]),
        }
        for l in range(2):
            m[f"wa{l}"] = f8c(was[l])
        # l2: prodT = adjT .* waT (transpose of this core's own matrices)
        m["waT"] = f8c(was[2].T)
        for l in range(3):
            m[f"w{l}c"] = f32c(wcs[l])
            m[f"w{l}b"] = f32c(inputs[f"l{l}_bias"])
        maps.append(m)
    return maps


_nc_cache = {}


def _get_program(n_cores, N):
    key = (n_cores, N)
    if key not in _nc_cache:
        _nc_cache[key] = build_program(n_cores, N)
    return _nc_cache[key]


def kernel(**inputs) -> np.ndarray:
    n_cores = 8
    nc = _get_program(n_cores, N_FULL)
    in_maps = make_in_maps(inputs, n_cores)
    res = run_bass_kernel_spmd(nc, in_maps, core_ids=list(range(n_cores)))
    out = np.zeros((B, 1), dtype=np.float32)
    for b in range(B):
        out[b, 0] = res.results[2 * b]["ypred"][0, 0]
    return out
